# revision 28
# baseline (speedup 1.0000x reference)
"""Trainium2 Bass kernel for nn_ChannelMixing (RWKV-style channel mixing).

Math: the reference's FFT decay-conv is the first-order IIR
    h[t] = mix*h[t-1] + x[t],  h[-1] = last_x/(1-mix)
and x_mix = (1-mix)*h, so with weights pre-scaled by (1-mix):
    k = h_k @ (Wk*(1-mix_k)).T,  r = h_r @ (Wr*(1-mix_r)).T
    out = sigmoid(r) * (relu(k)^2 @ Wv.T)

Sharding: time dimension L=4096 split over 8 cores (512 rows each) with a
64-step halo to warm up the scan state (decay <= sigmoid(1) ~ 0.731, so
carry across 64 steps < 3e-9 — below the tolerance). Core 0 gets the exact
initial state via a per-core init column; no collectives.

v2 layout: all matmul operands bf16 (halves the 50 MB/core weight DMA that
made v1 DMA-bound; scan state stays fp32 internally, only h is rounded).
The three GEMMs run as 48 16-deep same-bank PSUM accumulation chains,
skewed 2 steps apart across the 8 banks so chain closures (and their
ACT-side evictions) stagger instead of stalling the PE at phase edges.
Weight tiles [128, 1024] bf16 stream through a small pool in exact
consumption order. xs is loaded once and shared by the k and r scans.
"""
import numpy as np
import ml_dtypes
from contextlib import ExitStack

import concourse.bass as bass
from concourse import bacc
import concourse.tile as tile
import concourse.mybir as mybir
from concourse.bass_utils import run_bass_kernel_spmd

LEN, DIM = 4096, 2048
NCORES = 8
P = 128
HALO = 64

f32 = mybir.dt.float32
bf16 = mybir.dt.bfloat16
fp8 = mybir.dt.float8e4
WRS = 128.0   # fp8 scale for Wr (keeps entries out of e4m3 subnormals)
Alu = mybir.AluOpType
Act = mybir.ActivationFunctionType
PM = mybir.MatmulPerfMode

_cache = {}


def _build(dim, tloc, halo):
    """Build + compile the per-core SPMD program."""
    nt = dim // P          # 16 channel tiles
    ts = tloc + halo       # time slab incl. halo
    NF = tloc              # 512: matmul moving size == psum bank
    SW = 2                 # skew (in steps) between adjacent psum banks

    nc = bacc.Bacc(trn_type="TRN2", debug=False)

    xs_d = nc.dram_tensor("xs", [dim, ts], bf16, kind="ExternalInput").ap()
    dec_d = nc.dram_tensor("dec", [P, 2 * nt], f32, kind="ExternalInput").ap()
    ini_d = nc.dram_tensor("ini", [P, 2 * nt], f32, kind="ExternalInput").ap()
    wk_d = nc.dram_tensor("wk", [dim, dim], bf16, kind="ExternalInput").ap()  # [i, o]
    # wr: fp8e4m3, host-paired for DoubleRow: row (kp*2+half)*128+p,
    # col j*1024+u  ->  Wr_scaled[(2kp+j)*128+p, half*1024+u] * 128
    wr_d = nc.dram_tensor("wr", [dim, dim], fp8, kind="ExternalInput").ap()
    wv_d = nc.dram_tensor("wv", [dim, dim], bf16, kind="ExternalInput").ap()  # [o1, o2]
    out_d = nc.dram_tensor("out", [dim, tloc], bf16, kind="ExternalOutput").ap()

    with tile.TileContext(nc) as tc, ExitStack() as ctx:
        const = ctx.enter_context(tc.tile_pool(name="const", bufs=1))
        xs_pool = ctx.enter_context(tc.tile_pool(name="xs", bufs=nt))
        h_pool = ctx.enter_context(tc.tile_pool(name="h", bufs=1))
        w_pool = ctx.enter_context(tc.tile_pool(name="w", bufs=34))
        w8_pool = ctx.enter_context(tc.tile_pool(name="w8", bufs=1))
        h8_pool = ctx.enter_context(tc.tile_pool(name="h8", bufs=1))
        hl_pool = ctx.enter_context(tc.tile_pool(name="hl", bufs=2))
        sq_pool = ctx.enter_context(tc.tile_pool(name="sq", bufs=1))
        sg_pool = ctx.enter_context(tc.tile_pool(name="sg", bufs=1))
        rr_pool = ctx.enter_context(tc.tile_pool(name="rr", bufs=2))
        o_pool = ctx.enter_context(tc.tile_pool(name="o", bufs=3))
        ps_pool = ctx.enter_context(tc.tile_pool(name="ps", bufs=1, space="PSUM"))


        # ---- weight + xs streams (sync queue), consumption order.
        # xs0 leads (gates scan0); xs1-5 interleave with wk half0 so the
        # scan cadence and the matmul chase both stay fed from t~7us.
        NXS = 9
        xs_t = [None] * nt
        wtiles = {}

        def load_xs(ct, eng):
            xs = xs_pool.tile([P, ts], bf16, tag="xs", name=f"xs{ct}")
            eng.dma_start(xs[:], xs_d[ct * P:(ct + 1) * P, :])
            xs_t[ct] = xs

        def load_w(X, half, k):
            wd = (wk_d, None, wv_d)[X]
            wt = w_pool.tile([P, 1024], bf16, tag="w", name=f"w{X}_{half}_{k}")
            nc.sync.dma_start(
                wt[:], wd[k * P:(k + 1) * P, half * 1024:(half + 1) * 1024])
            wtiles[(X, half, k)] = wt

        def load_w8(half, kp):
            wt = w8_pool.tile([P, 2, 1024], fp8, tag=f"w8_{half}_{kp}",
                              name=f"w8_{half}_{kp}")
            nc.sync.dma_start(
                wt[:], wr_d[(kp * 2 + half) * P:(kp * 2 + half + 1) * P, :])
            wtiles[(1, half, kp)] = wt

        load_xs(0, nc.sync)
        dec_t = const.tile([P, 2 * nt], f32)
        nc.sync.dma_start(dec_t[:], dec_d)
        ini_t = const.tile([P, 2 * nt], f32)
        nc.sync.dma_start(ini_t[:], ini_d)
        # two xs per two wk tiles: scans stay fed (~1.08us/tile < the
        # ~1.2-1.33us DVE scan cadence) while wk000 still lands early.
        load_xs(1, nc.sync)
        load_xs(2, nc.sync)
        wk_next = 0
        for ct in range(3, NXS, 2):
            load_w(0, 0, wk_next)
            load_w(0, 0, wk_next + 1)
            wk_next += 2
            load_xs(ct, nc.sync)
            if ct + 1 < NXS:
                load_xs(ct + 1, nc.sync)
        for k in range(wk_next, nt):
            load_w(0, 0, k)
        for k in range(nt):
            load_w(0, 1, k)
        for half in range(2):
            for kp in range(nt // 2):
                load_w8(half, kp)
        for half in range(2):
            for k in range(nt):
                load_w(2, half, k)

        # ---- scans (DVE): xs loaded once, k then r path ----
        for ct in range(NXS, nt):
            load_xs(ct, nc.scalar)
        # k-path: one scan per tile, bf16 out (halo inline).
        hk = [None] * nt
        for ct in range(nt):
            hs = h_pool.tile([P, ts], bf16, tag=f"h0_{ct}", name=f"h0_{ct}")
            nc.vector.tensor_tensor_scan(
                hs[:],
                dec_t[:, 2 * ct: 2 * ct + 1].broadcast_to([P, ts]),
                xs_t[ct][:],
                ini_t[:, 2 * ct: 2 * ct + 1],
                op0=Alu.mult, op1=Alu.add)
            hk[ct] = hs
        # r-path: halo scan (f32 scratch) chains into a body scan that
        # writes the fp8 DoubleRow pair tile hr8[ct//2][:, ct%2, :].
        hr8 = [h8_pool.tile([P, 2, tloc], fp8, tag=f"h8_{kp}", name=f"h8_{kp}")
               for kp in range(nt // 2)]
        for ct in range(nt):
            dcol = dec_t[:, 2 * ct + 1: 2 * ct + 2]
            hl = hl_pool.tile([P, halo], f32, tag="hl", name=f"hl{ct}")
            nc.vector.tensor_tensor_scan(
                hl[:], dcol.broadcast_to([P, halo]), xs_t[ct][:, 0:halo],
                ini_t[:, 2 * ct + 1: 2 * ct + 2], op0=Alu.mult, op1=Alu.add)
            nc.vector.tensor_tensor_scan(
                hr8[ct // 2][:, ct % 2, :], dcol.broadcast_to([P, tloc]),
                xs_t[ct][:, halo:], hl[:, halo - 1: halo],
                op0=Alu.mult, op1=Alu.add)

        sq = [None] * nt   # relu(k)^2, bf16 [P, NF], chan-major
        sig = [None] * nt  # sigmoid(r), bf16 [P, NF]

        # ---- PE warmup: ramp p-state on a memset tile (no DMA dependency,
        # runs right after the preamble); results are discarded.
        wm_t = const.tile([P, NF], bf16, name="wm")
        nc.gpsimd.memset(wm_t[:], 0)
        ps_w = ps_pool.tile([P, NF], f32, tag="b7", name="ps_warm")
        for _ in range(16):
            nc.tensor.matmul(ps_w[:], wm_t[:, 0:P], wm_t[:],
                             start=True, stop=True)

        # ---- 48 accumulation chains over 8 psum banks ----
        # chain jj: weight X=jj//16, half=(jj%16)//8, bank=jj%8,
        # o-group g=half*2+(jj%8)//4, m-tile m=jj%4 -> o-tile oidx=g*4+m.
        # Chains 0-7 (k half0) run kt-outer, 8 matmuls per scan tile, so the
        # PE chases the scan frontier densely (1.84us/step vs 1.41us/scan).
        # Chains 8-47 run as serial same-bank 16-chains: closures stagger
        # 3.65us apart, so ACT evictions never gate psum-bank reuse.
        ps_t = [None] * 48

        def mm(jj, kt):
            X = jj // 16
            half = (jj % 16) // 8
            coff = ((jj % 8) // 4) * 512 + (jj % 4) * P
            wt = wtiles[(X, half, kt)]
            if kt == 0:
                ps_t[jj] = ps_pool.tile([P, NF], f32, tag=f"b{jj % 8}",
                                        name=f"ps{jj}")
            if X == 1:
                # fp8 DoubleRow: two k-subtiles per step, 8 steps
                nc.tensor.matmul(ps_t[jj][:], wt[:, :, coff:coff + P],
                                 hr8[kt][:], start=(kt == 0),
                                 stop=(kt == nt // 2 - 1),
                                 perf_mode=PM.DoubleRow)
                return
            if X == 0:
                rhs = hk[kt][:, halo:halo + NF]
            else:
                rhs = sq[kt][:]
            nc.tensor.matmul(ps_t[jj][:], wt[:, coff:coff + P], rhs,
                             start=(kt == 0), stop=(kt == nt - 1))

        def evict(jj):
            X = jj // 16
            oidx = ((jj % 16) // 8) * 8 + ((jj % 8) // 4) * 4 + jj % 4
            psum = ps_t[jj]
            if X == 0:
                rr = rr_pool.tile([P, NF], f32, tag="rr", name=f"rr{jj}")
                nc.scalar.activation(rr[:], psum[:], Act.Relu)
                sq[oidx] = sq_pool.tile([P, NF], bf16, tag=f"sq{oidx}",
                                        name=f"sq{oidx}")
                nc.scalar.activation(sq[oidx][:], rr[:], Act.Square)
            elif X == 1:
                sig[oidx] = sg_pool.tile([P, NF], bf16, tag=f"sg{oidx}",
                                         name=f"sg{oidx}")
                nc.scalar.activation(sig[oidx][:], psum[:], Act.Sigmoid,
                                     scale=1.0 / WRS)
            else:
                ot = o_pool.tile([P, NF], bf16, tag="ot", name=f"ot{jj}")
                nc.vector.tensor_mul(ot[:], psum[:], sig[oidx][:])
                nc.scalar.dma_start(out_d[oidx * P:(oidx + 1) * P, :], ot[:])

        for kt in range(nt):
            for jj in range(8):
                mm(jj, kt)
        for jj in range(8):
            evict(jj)
        for jj in range(8, 48):
            nkt = nt // 2 if jj // 16 == 1 else nt
            for kt in range(nkt):
                mm(jj, kt)
            evict(jj)

    nc.compile()
    return nc


def _sigmoid(v):
    return 1.0 / (1.0 + np.exp(-v.astype(np.float64)))


def _prep(x, Wk, Wr, Wv, mix_k, mix_r, lxk, lxr, ncores, halo):
    """Host-side prep: transposes, weight pre-scaling, per-core slabs."""
    dim = x.shape[1]
    tloc = x.shape[0] // ncores
    mk = _sigmoid(mix_k).astype(np.float32)
    mr = _sigmoid(mix_r).astype(np.float32)
    h0k = (lxk / (1.0 - mk)).astype(np.float32)
    h0r = (lxr / (1.0 - mr)).astype(np.float32)
    nt = dim // P
    dec = np.empty((P, 2 * nt), np.float32)   # SBUF image: [p, 2*ct+path]
    dec[:, 0::2] = mk.reshape(nt, P).T
    dec[:, 1::2] = mr.reshape(nt, P).T

    bf = ml_dtypes.bfloat16
    wk = np.ascontiguousarray((Wk * (1.0 - mk)[None, :]).T).astype(bf)
    wv = np.ascontiguousarray(Wv.T).astype(bf)
    # Wr: fp8 DoubleRow pairing. wr8[(kp*2+half)*128+p, j*1024+u]
    #   = Wr_s[(2kp+j)*128+p, half*1024+u] * WRS
    wr_s = (Wr * (1.0 - mr)[None, :]).T * np.float32(128.0)
    A = wr_s.reshape(dim // 256, 2, P, 2, dim // 2)
    wr = np.ascontiguousarray(
        A.transpose(0, 3, 2, 1, 4).reshape(dim, dim)
    ).astype(ml_dtypes.float8_e4m3fn)

    xT = np.ascontiguousarray(x.T.astype(np.float32))       # [dim, L]
    in_maps = []
    for c in range(ncores):
        t0 = c * tloc
        slab = np.empty((dim, halo + tloc), ml_dtypes.bfloat16)
        if c == 0:
            slab[:, :halo] = ml_dtypes.bfloat16(0.0)
            bk = (h0k.astype(np.float64) * (1.0 / mk.astype(np.float64)) ** halo
                  ).astype(np.float32)
            br = (h0r.astype(np.float64) * (1.0 / mr.astype(np.float64)) ** halo
                  ).astype(np.float32)
            ini = np.empty((P, 2 * nt), np.float32)
            ini[:, 0::2] = bk.reshape(nt, P).T
            ini[:, 1::2] = br.reshape(nt, P).T
        else:
            slab[:, :halo] = xT[:, t0 - halo: t0]
            ini = np.zeros((P, 2 * nt), np.float32)
        slab[:, halo:] = xT[:, t0: t0 + tloc]
        in_maps.append({
            "xs": slab, "dec": dec, "ini": np.ascontiguousarray(ini),
            "wk": wk, "wr": wr, "wv": wv,
        })
    return in_maps


def kernel(x, Wk, Wr, Wv, mix_k, mix_r, last_x_mix_k, last_x_mix_r):
    x = np.asarray(x, np.float32)
    Wk = np.asarray(Wk, np.float32)
    Wr = np.asarray(Wr, np.float32)
    Wv = np.asarray(Wv, np.float32)
    mix_k = np.asarray(mix_k, np.float32)
    mix_r = np.asarray(mix_r, np.float32)
    lxk = np.asarray(last_x_mix_k, np.float32)
    lxr = np.asarray(last_x_mix_r, np.float32)

    L, dim = x.shape
    tloc = L // NCORES
    key = (dim, tloc, HALO)
    if key not in _cache:
        _cache[key] = _build(dim, tloc, HALO)
    nc = _cache[key]

    in_maps = _prep(x, Wk, Wr, Wv, mix_k, mix_r, lxk, lxr, NCORES, HALO)
    # First execution on a cold device occasionally returns
    # NRT_EXEC_UNIT_UNRECOVERABLE; a retry has always succeeded.
    res = None
    for attempt in range(3):
        try:
            res = run_bass_kernel_spmd(nc, in_maps, core_ids=list(range(NCORES)))
            break
        except Exception:
            if attempt == 2:
                raise

    out = np.empty((L, dim), np.float32)
    for c in range(NCORES):
        out[c * tloc:(c + 1) * tloc, :] = res.results[c]["out"].astype(np.float32).T
    return out


# revision 30
# speedup vs baseline: 1.0030x; 1.0030x over previous
"""Trainium2 Bass kernel for nn_ChannelMixing (RWKV-style channel mixing).

Math: the reference's FFT decay-conv is the first-order IIR
    h[t] = mix*h[t-1] + x[t],  h[-1] = last_x/(1-mix)
and x_mix = (1-mix)*h, so with weights pre-scaled by (1-mix):
    k = h_k @ (Wk*(1-mix_k)).T,  r = h_r @ (Wr*(1-mix_r)).T
    out = sigmoid(r) * (relu(k)^2 @ Wv.T)

Sharding: time dimension L=4096 split over 8 cores (512 rows each) with a
64-step halo to warm up the scan state (decay <= sigmoid(1) ~ 0.731, so
carry across 64 steps < 3e-9 — below the tolerance). Core 0 gets the exact
initial state via a per-core init column; no collectives.

Dtypes: the v1 kernel was DMA-bound (64 MB/core at ~300 GB/s). Now Wk/Wv,
h_k, sq, sig, xs and out are bf16 and Wr/h_r are fp8e4m3 (scaled x128 to
dodge subnormals; the r-GEMM error is squashed by the sigmoid gate —
measured 7e-3 L2 vs the 2e-2 gate). DMA is ~25 MB/core, fully hidden.

PE schedule (the binding resource, ~143 us busy of ~167 us span):
- k half0: 8 accumulation chains run kt-outer so the PE chases the scan
  frontier densely (8 matmuls per scan tile; scans pipeline at ~1.2-1.4 us
  vs 1.84 us of matmul per tile, so the PE stays fed).
- everything else: serial same-bank 16-deep (bf16) / 8-deep (fp8
  DoubleRow) chains at the 216 ns/matmul hardware cadence; closures
  self-stagger so ACT-side evictions never gate PSUM bank reuse.
- r-GEMM: fp8 DoubleRow (2 k-subtiles per instruction at 0.5 cycles/row),
  operands host-paired as [128, 2, free]; the r decay-scan splits into a
  fp32 halo scan chained into a body scan that writes the fp8 pair tile.
- PE warmup on a memset tile (no DMA dependency) ramps the p-state before
  the first weight tile lands.
Weight tiles [128, 1024] stream through a pool sized to hold two halves
(the serial chains re-read each half for ~28 us); xs rides ahead of the
weights on the sync DMA queue; evictions: relu+square on ACT (k), scaled
sigmoid on ACT (r), gate-mul on DVE + DMA-out (v).
"""
import numpy as np
import ml_dtypes
from contextlib import ExitStack

import concourse.bass as bass
from concourse import bacc
import concourse.tile as tile
import concourse.mybir as mybir
from concourse.bass_utils import run_bass_kernel_spmd

LEN, DIM = 4096, 2048
NCORES = 8
P = 128
HALO = 64

f32 = mybir.dt.float32
bf16 = mybir.dt.bfloat16
fp8 = mybir.dt.float8e4
WRS = 128.0   # fp8 scale for Wr (keeps entries out of e4m3 subnormals)
Alu = mybir.AluOpType
Act = mybir.ActivationFunctionType
PM = mybir.MatmulPerfMode

_cache = {}


def _build(dim, tloc, halo):
    """Build + compile the per-core SPMD program."""
    nt = dim // P          # 16 channel tiles
    ts = tloc + halo       # time slab incl. halo
    NF = tloc              # 512: matmul moving size == psum bank
    SW = 2                 # skew (in steps) between adjacent psum banks

    nc = bacc.Bacc(trn_type="TRN2", debug=False)

    xs_d = nc.dram_tensor("xs", [dim, ts], bf16, kind="ExternalInput").ap()
    dec_d = nc.dram_tensor("dec", [P, 2 * nt], f32, kind="ExternalInput").ap()
    ini_d = nc.dram_tensor("ini", [P, 2 * nt], f32, kind="ExternalInput").ap()
    wk_d = nc.dram_tensor("wk", [dim, dim], bf16, kind="ExternalInput").ap()  # [i, o]
    # wr: fp8e4m3, host-paired for DoubleRow: row (kp*2+half)*128+p,
    # col j*1024+u  ->  Wr_scaled[(2kp+j)*128+p, half*1024+u] * 128
    wr_d = nc.dram_tensor("wr", [dim, dim], fp8, kind="ExternalInput").ap()
    wv_d = nc.dram_tensor("wv", [dim, dim], bf16, kind="ExternalInput").ap()  # [o1, o2]
    out_d = nc.dram_tensor("out", [dim, tloc], bf16, kind="ExternalOutput").ap()

    with tile.TileContext(nc) as tc, ExitStack() as ctx:
        const = ctx.enter_context(tc.tile_pool(name="const", bufs=1))
        xs_pool = ctx.enter_context(tc.tile_pool(name="xs", bufs=nt))
        h_pool = ctx.enter_context(tc.tile_pool(name="h", bufs=1))
        w_pool = ctx.enter_context(tc.tile_pool(name="w", bufs=34))
        w8_pool = ctx.enter_context(tc.tile_pool(name="w8", bufs=1))
        h8_pool = ctx.enter_context(tc.tile_pool(name="h8", bufs=1))
        hl_pool = ctx.enter_context(tc.tile_pool(name="hl", bufs=2))
        sq_pool = ctx.enter_context(tc.tile_pool(name="sq", bufs=1))
        sg_pool = ctx.enter_context(tc.tile_pool(name="sg", bufs=1))
        rr_pool = ctx.enter_context(tc.tile_pool(name="rr", bufs=2))
        o_pool = ctx.enter_context(tc.tile_pool(name="o", bufs=3))
        ps_pool = ctx.enter_context(tc.tile_pool(name="ps", bufs=1, space="PSUM"))


        # ---- weight + xs streams (sync queue), consumption order.
        # xs0 leads (gates scan0); xs1-5 interleave with wk half0 so the
        # scan cadence and the matmul chase both stay fed from t~7us.
        NXS = 6
        xs_t = [None] * nt
        wtiles = {}

        def load_xs(ct, eng):
            xs = xs_pool.tile([P, ts], bf16, tag="xs", name=f"xs{ct}")
            eng.dma_start(xs[:], xs_d[ct * P:(ct + 1) * P, :])
            xs_t[ct] = xs

        def load_w(X, half, k):
            wd = (wk_d, None, wv_d)[X]
            wt = w_pool.tile([P, 1024], bf16, tag="w", name=f"w{X}_{half}_{k}")
            nc.sync.dma_start(
                wt[:], wd[k * P:(k + 1) * P, half * 1024:(half + 1) * 1024])
            wtiles[(X, half, k)] = wt

        def load_w8(half, kp):
            wt = w8_pool.tile([P, 2, 1024], fp8, tag=f"w8_{half}_{kp}",
                              name=f"w8_{half}_{kp}")
            nc.sync.dma_start(
                wt[:], wr_d[(kp * 2 + half) * P:(kp * 2 + half + 1) * P, :])
            wtiles[(1, half, kp)] = wt

        load_xs(0, nc.sync)
        dec_t = const.tile([P, 2 * nt], f32)
        nc.sync.dma_start(dec_t[:], dec_d)
        ini_t = const.tile([P, 2 * nt], f32)
        nc.sync.dma_start(ini_t[:], ini_d)
        load_w(0, 0, 0)
        load_w(0, 0, 1)
        for ct in range(1, NXS):
            load_xs(ct, nc.sync)
            load_w(0, 0, ct + 1)
        for k in range(NXS + 1, nt):
            load_w(0, 0, k)
        for k in range(nt):
            load_w(0, 1, k)
        for half in range(2):
            for kp in range(nt // 2):
                load_w8(half, kp)
        for half in range(2):
            for k in range(nt):
                load_w(2, half, k)

        # ---- scans (DVE): xs loaded once, k then r path ----
        for ct in range(NXS, nt):
            load_xs(ct, nc.scalar)
        # k-path: one scan per tile, bf16 out (halo inline).
        hk = [None] * nt
        for ct in range(nt):
            hs = h_pool.tile([P, ts], bf16, tag=f"h0_{ct}", name=f"h0_{ct}")
            nc.vector.tensor_tensor_scan(
                hs[:],
                dec_t[:, 2 * ct: 2 * ct + 1].broadcast_to([P, ts]),
                xs_t[ct][:],
                ini_t[:, 2 * ct: 2 * ct + 1],
                op0=Alu.mult, op1=Alu.add)
            hk[ct] = hs
        # r-path: halo scan (f32 scratch) chains into a body scan that
        # writes the fp8 DoubleRow pair tile hr8[ct//2][:, ct%2, :].
        hr8 = [h8_pool.tile([P, 2, tloc], fp8, tag=f"h8_{kp}", name=f"h8_{kp}")
               for kp in range(nt // 2)]
        for ct in range(nt):
            dcol = dec_t[:, 2 * ct + 1: 2 * ct + 2]
            hl = hl_pool.tile([P, halo], f32, tag="hl", name=f"hl{ct}")
            nc.vector.tensor_tensor_scan(
                hl[:], dcol.broadcast_to([P, halo]), xs_t[ct][:, 0:halo],
                ini_t[:, 2 * ct + 1: 2 * ct + 2], op0=Alu.mult, op1=Alu.add)
            nc.vector.tensor_tensor_scan(
                hr8[ct // 2][:, ct % 2, :], dcol.broadcast_to([P, tloc]),
                xs_t[ct][:, halo:], hl[:, halo - 1: halo],
                op0=Alu.mult, op1=Alu.add)

        sq = [None] * nt   # relu(k)^2, bf16 [P, NF], chan-major
        sig = [None] * nt  # sigmoid(r), bf16 [P, NF]

        # ---- PE warmup: ramp p-state on a memset tile (no DMA dependency,
        # runs right after the preamble); results are discarded.
        wm_t = const.tile([P, NF], bf16, name="wm")
        nc.gpsimd.memset(wm_t[:], 0)
        ps_w = ps_pool.tile([P, NF], f32, tag="b7", name="ps_warm")
        for _ in range(24):
            nc.tensor.matmul(ps_w[:], wm_t[:, 0:P], wm_t[:],
                             start=True, stop=True)

        # ---- 48 accumulation chains over 8 psum banks ----
        # chain jj: weight X=jj//16, half=(jj%16)//8, bank=jj%8,
        # o-group g=half*2+(jj%8)//4, m-tile m=jj%4 -> o-tile oidx=g*4+m.
        # Chains 0-7 (k half0) run kt-outer, 8 matmuls per scan tile, so the
        # PE chases the scan frontier densely (1.84us/step vs 1.41us/scan).
        # Chains 8-47 run as serial same-bank 16-chains: closures stagger
        # 3.65us apart, so ACT evictions never gate psum-bank reuse.
        ps_t = [None] * 48

        def mm(jj, kt):
            X = jj // 16
            half = (jj % 16) // 8
            coff = ((jj % 8) // 4) * 512 + (jj % 4) * P
            wt = wtiles[(X, half, kt)]
            if kt == 0:
                ps_t[jj] = ps_pool.tile([P, NF], f32, tag=f"b{jj % 8}",
                                        name=f"ps{jj}")
            if X == 1:
                # fp8 DoubleRow: two k-subtiles per step, 8 steps
                nc.tensor.matmul(ps_t[jj][:], wt[:, :, coff:coff + P],
                                 hr8[kt][:], start=(kt == 0),
                                 stop=(kt == nt // 2 - 1),
                                 perf_mode=PM.DoubleRow)
                return
            if X == 0:
                rhs = hk[kt][:, halo:halo + NF]
            else:
                rhs = sq[kt][:]
            nc.tensor.matmul(ps_t[jj][:], wt[:, coff:coff + P], rhs,
                             start=(kt == 0), stop=(kt == nt - 1))

        def evict(jj):
            X = jj // 16
            oidx = ((jj % 16) // 8) * 8 + ((jj % 8) // 4) * 4 + jj % 4
            psum = ps_t[jj]
            if X == 0:
                rr = rr_pool.tile([P, NF], f32, tag="rr", name=f"rr{jj}")
                nc.scalar.activation(rr[:], psum[:], Act.Relu)
                sq[oidx] = sq_pool.tile([P, NF], bf16, tag=f"sq{oidx}",
                                        name=f"sq{oidx}")
                nc.scalar.activation(sq[oidx][:], rr[:], Act.Square)
            elif X == 1:
                sig[oidx] = sg_pool.tile([P, NF], bf16, tag=f"sg{oidx}",
                                         name=f"sg{oidx}")
                nc.scalar.activation(sig[oidx][:], psum[:], Act.Sigmoid,
                                     scale=1.0 / WRS)
            else:
                ot = o_pool.tile([P, NF], bf16, tag="ot", name=f"ot{jj}")
                nc.vector.tensor_mul(ot[:], psum[:], sig[oidx][:])
                nc.scalar.dma_start(out_d[oidx * P:(oidx + 1) * P, :], ot[:])

        for kt in range(nt):
            for jj in range(8):
                mm(jj, kt)
        for jj in range(8):
            evict(jj)
        for jj in range(8, 48):
            nkt = nt // 2 if jj // 16 == 1 else nt
            for kt in range(nkt):
                mm(jj, kt)
            evict(jj)

    nc.compile()
    return nc


def _sigmoid(v):
    return 1.0 / (1.0 + np.exp(-v.astype(np.float64)))


def _prep(x, Wk, Wr, Wv, mix_k, mix_r, lxk, lxr, ncores, halo):
    """Host-side prep: transposes, weight pre-scaling, per-core slabs."""
    dim = x.shape[1]
    tloc = x.shape[0] // ncores
    mk = _sigmoid(mix_k).astype(np.float32)
    mr = _sigmoid(mix_r).astype(np.float32)
    h0k = (lxk / (1.0 - mk)).astype(np.float32)
    h0r = (lxr / (1.0 - mr)).astype(np.float32)
    nt = dim // P
    dec = np.empty((P, 2 * nt), np.float32)   # SBUF image: [p, 2*ct+path]
    dec[:, 0::2] = mk.reshape(nt, P).T
    dec[:, 1::2] = mr.reshape(nt, P).T

    bf = ml_dtypes.bfloat16
    wk = np.ascontiguousarray((Wk * (1.0 - mk)[None, :]).T).astype(bf)
    wv = np.ascontiguousarray(Wv.T).astype(bf)
    # Wr: fp8 DoubleRow pairing. wr8[(kp*2+half)*128+p, j*1024+u]
    #   = Wr_s[(2kp+j)*128+p, half*1024+u] * WRS
    wr_s = (Wr * (1.0 - mr)[None, :]).T * np.float32(128.0)
    A = wr_s.reshape(dim // 256, 2, P, 2, dim // 2)
    wr = np.ascontiguousarray(
        A.transpose(0, 3, 2, 1, 4).reshape(dim, dim)
    ).astype(ml_dtypes.float8_e4m3fn)

    xT = np.ascontiguousarray(x.T.astype(np.float32))       # [dim, L]
    in_maps = []
    for c in range(ncores):
        t0 = c * tloc
        slab = np.empty((dim, halo + tloc), ml_dtypes.bfloat16)
        if c == 0:
            slab[:, :halo] = ml_dtypes.bfloat16(0.0)
            bk = (h0k.astype(np.float64) * (1.0 / mk.astype(np.float64)) ** halo
                  ).astype(np.float32)
            br = (h0r.astype(np.float64) * (1.0 / mr.astype(np.float64)) ** halo
                  ).astype(np.float32)
            ini = np.empty((P, 2 * nt), np.float32)
            ini[:, 0::2] = bk.reshape(nt, P).T
            ini[:, 1::2] = br.reshape(nt, P).T
        else:
            slab[:, :halo] = xT[:, t0 - halo: t0]
            ini = np.zeros((P, 2 * nt), np.float32)
        slab[:, halo:] = xT[:, t0: t0 + tloc]
        in_maps.append({
            "xs": slab, "dec": dec, "ini": np.ascontiguousarray(ini),
            "wk": wk, "wr": wr, "wv": wv,
        })
    return in_maps


def kernel(x, Wk, Wr, Wv, mix_k, mix_r, last_x_mix_k, last_x_mix_r):
    x = np.asarray(x, np.float32)
    Wk = np.asarray(Wk, np.float32)
    Wr = np.asarray(Wr, np.float32)
    Wv = np.asarray(Wv, np.float32)
    mix_k = np.asarray(mix_k, np.float32)
    mix_r = np.asarray(mix_r, np.float32)
    lxk = np.asarray(last_x_mix_k, np.float32)
    lxr = np.asarray(last_x_mix_r, np.float32)

    L, dim = x.shape
    tloc = L // NCORES
    key = (dim, tloc, HALO)
    if key not in _cache:
        _cache[key] = _build(dim, tloc, HALO)
    nc = _cache[key]

    in_maps = _prep(x, Wk, Wr, Wv, mix_k, mix_r, lxk, lxr, NCORES, HALO)
    # First execution on a cold device occasionally returns
    # NRT_EXEC_UNIT_UNRECOVERABLE; a retry has always succeeded.
    res = None
    for attempt in range(3):
        try:
            res = run_bass_kernel_spmd(nc, in_maps, core_ids=list(range(NCORES)))
            break
        except Exception:
            if attempt == 2:
                raise

    out = np.empty((L, dim), np.float32)
    for c in range(NCORES):
        out[c * tloc:(c + 1) * tloc, :] = res.results[c]["out"].astype(np.float32).T
    return out
